# revision 1
# baseline (speedup 1.0000x reference)
"""nn_CRFLayer: CRF Viterbi decode on 8 Trainium2 NeuronCores.

Data parallel over batch: each core decodes 64 of the 512 sequences.
Self-contained: hardcodes B=512, T=512, D=48, n_cores=8.

Design (v2): the forward pass stores NO backpointers — only the alpha
history — and the backward pass recomputes one 48-wide score row per
(b, t) to recover each needed backpointer exactly.

  Forward (all DVE, 3 ops/step): one custom fused DVE op (CRF_SEGMAX_ADD,
  a segmented max-scan that resets at each 48-element page boundary)
  computes the whole tropical mat-vec max_prev(trans + alpha) in a single
  1152-elem pass; the segment maxima sit at column 47 and are read
  strided by the emit add; a pair-swap stream_shuffle rebuilds the full
  alpha row across partition pairs. Layout: partitions = (b, half)
  interleaved (p = 2b+ch), 24 cur x 48 prev per partition, prev rotated
  per half with pre-rotated trans. Alpha evolves freely through padding
  (no freeze needed: at any padded step the backward's candidate row
  degenerates to alpha itself, whose argmax re-syncs the tag chain to
  last_tag at t = L-1). Alpha history streams to DRAM in the background.

  Backward (single merged pipeline, ~8 instructions per step): gather
  trans[:, tag] for all 64 sequences at once — one PE transpose of the
  [64, 48] one-hot against a [64, 64] identity into a [48, 64] PSUM
  tile, one ACT copy to SBUF, then 3 accumulated bf16 matmuls against
  an exact 3-way bf16 split of transT (fp32 PE matmuls are NOT bitwise
  exact; 3 x 8 mantissa bits are) — then two custom fused DVE ops:
  cand = tsel*valid + alpha with fused row max, and first-argmax via
  (cand >= maxr) * (iota-64) with fused min-reduce. All adds are
  bitwise-identical to the forward, ties take the first index, and the
  decoded tags match the fp32 reference bitwise. Every custom-DVE
  operand sits at base partition 0 (custom DVE ops drop nonzero base
  partitions on HW), and each PE transpose owns a whole PSUM tile
  (sharing one tile between transposes crashes the device).
"""

import numpy as np

import concourse.bass as bass
import concourse.mybir as mybir
from concourse.tile import TileContext
from concourse.tile_rust import add_dep_helper


# --- runtime-registered fused DVE ops (standard Spec DSL, lowered by the
# --- production table generator; sha self-pinned at registration) ----------
def _register_custom_ops():
    import concourse.dve_ops as dvo
    from concourse.dve_spec import (
        Spec, Src0, Src1, C0, C1, maxx, minn, _has_src1, lower as dve_lower,
    )
    from concourse.dve_uop import DveOpSpec

    def reg(name, spec, subdim=False):
        if name in dvo.CUSTOM_DVE_SPECS:
            return next(op for op in dvo.OPS if op.name == name)
        row = dvo._CUSTOM_DVE_ROW_BASE + len(dvo.OPS)
        shas = {}
        for ver in ("v3", "v4"):
            s = DveOpSpec(name=name, opcode=row, uops=dve_lower(spec, ver=ver),
                          rd1_en=_has_src1(spec))
            shas[ver] = s.sha(ver)
        op = dvo.DveOp(name, spec, subdim=subdim, uops_sha=shas)
        dvo.OPS.append(op)
        dvo.CUSTOM_DVE_SPECS[name] = spec
        dvo._SUB_OPCODE_FOR_NAME[name] = row
        return op

    import concourse.dve_spec as dsp
    from concourse.dve_spec import AluOp, Bin, MaxNeg, Scan

    # Segmented max-scan: a Scan whose accumulator RESETS at each subdim
    # (page) boundary. The DSL's page-counter machinery already emits a
    # one-element "step" state at SUB_DIM_DONE; we patch its override (for
    # scans marked with the sentinel _subdim_step=MaxNeg only) from
    # `op(CURR, step)` to `BYPASS(expr)` — i.e. state := current element.
    if not getattr(dsp, "_crf_segscan_patched", False):
        _orig_overrides = dsp._scan_overrides

        def _patched(scans, node_stage):
            seed, step = _orig_overrides(scans, node_stage)
            for sc in scans:
                if sc._subdim_step is dsp.MaxNeg:
                    step[node_stage[sc]] = dsp._Stage(dsp.AluOp.BYPASS, sc.expr)
            return seed, step

        dsp._scan_overrides = _patched

        # the body stage for a _subdim_step scan is lowered as
        # BYPASS(CURR) (PageIdx "hold within page"); our sentinel-marked
        # scan must instead combine normally: op(CURR, expr)
        _orig_nas = dsp._node_as_stage

        def _patched_nas(e):
            if isinstance(e, dsp.Scan) and e._subdim_step is dsp.MaxNeg:
                from concourse.dve_uop import AluInp
                return dsp._Stage(e.op, AluInp.CURR_ALU_OUT, e.expr)
            return _orig_nas(e)

        dsp._node_as_stage = _patched_nas
        dsp._crf_segscan_patched = True

    def _ref_segmax(in0, in1, c0, c1, c2):
        s = (in0.astype(np.float32) + in1).astype(np.float32)   # [P, S, N]
        return np.maximum.accumulate(s, axis=-1)

    segscan = Scan(AluOp.MAX, Bin(AluOp.ADD, dsp.Src0, dsp.Src1),
                   _subdim_step=MaxNeg)

    def _ref_cand_max(in0, in1, c0, c1, c2):
        b = (in0.astype(np.float32) * c0 + in1).astype(np.float32)
        return b, b.reshape(b.shape[0], -1).max(axis=-1, keepdims=True)

    def _ref_selmin(in0, in1, c0, c1, c2):
        b = ((in0 >= c0).astype(np.float32) * in1).astype(np.float32)
        return b, np.minimum(np.float32(c1) if np.isscalar(c1) else c1,
                             b.reshape(b.shape[0], -1).min(axis=-1, keepdims=True))

    # out = in0*s0 + in1 ; accum_out = max(out)   (cand row + its max)
    cand_max = reg("CRF_CAND_MAX", Spec(
        body=Src0 * C0 + Src1, accum=maxx, reference=_ref_cand_max,
    ))
    # out = (in0 >= s0) * in1 ; accum_out = min(s1, min(out))  (first argmax - 64)
    selmin = reg("CRF_SELMIN", Spec(
        body=(Src0 >= C0) * Src1, accum=minn, accum_init=C1,
        reference=_ref_selmin,
    ))
    # out[p, s, :] = running max of (in0 + in1) within each page s
    segmax = reg("CRF_SEGMAX_ADD", Spec(
        body=segscan, reference=_ref_segmax,
    ), subdim=True)
    return cand_max, selmin, segmax


CRF_CAND_MAX, CRF_SELMIN, CRF_SEGMAX_ADD = _register_custom_ops()

AL = mybir.AluOpType
F32 = mybir.dt.float32
BF16 = mybir.dt.bfloat16
U8 = mybir.dt.uint8
I32 = mybir.dt.int32

D = 48
HALF = 24
BLOC = 64
BIG = 64.0

PAIR_SWAP_MASK = [i ^ 1 for i in range(32)]

B = 512
T = 512
N_CORES = 8


def make_consts(trans: np.ndarray, T: int = T) -> dict[str, np.ndarray]:
    """Host-prepared constant tensors (derived from trans + shapes only)."""
    trans = np.asarray(trans, dtype=np.float32)
    trans_rep = np.zeros((128, HALF, D), dtype=np.float32)
    for ch in range(2):
        prev = (np.arange(D) + HALF * ch) % D
        cur = HALF * ch + np.arange(HALF)
        block = trans[prev][:, cur].T  # [cur24, prev48] in rotated prev order
        for b in range(BLOC):
            trans_rep[2 * b + ch] = block
    iota48 = np.arange(D, dtype=np.float32)
    # exact 3-way bf16 split of transT (24 mantissa bits = 3 x 8): the PE
    # gather accumulates the three pieces in fp32 PSUM, reconstructing
    # trans[prev, tag] bitwise-exactly (fp32 PE matmul is NOT exact).
    import ml_dtypes
    tT = np.ascontiguousarray(trans.T)
    bf = lambda v: v.astype(ml_dtypes.bfloat16)
    p0 = bf(tT)
    p1 = bf(tT - p0.astype(np.float32))
    p2 = bf(tT - p0.astype(np.float32) - p1.astype(np.float32))
    transT3 = np.concatenate([p0, p1, p2], axis=1)                   # [48, 144] bf16
    return {
        "trans_rep": trans_rep.reshape(128, HALF * D),
        "transT3": transT3,
        "ident64": np.eye(BLOC, dtype=ml_dtypes.bfloat16),
        "iota_m64": np.broadcast_to(iota48 - BIG, (BLOC, D)).copy(),
        "iota_big": np.broadcast_to(iota48 + BIG, (BLOC, D)).copy(),
    }


def make_core_inputs(logits_core, sent_lengths_core, consts) -> dict[str, np.ndarray]:
    L = np.asarray(sent_lengths_core, dtype=np.float32)
    lg = np.asarray(logits_core, dtype=np.float32)
    Tv = lg.shape[1]
    lg_il = lg.reshape(BLOC, Tv, 2, HALF).transpose(0, 2, 1, 3).reshape(128, Tv, HALF)
    ts = np.arange(Tv, dtype=np.float32)
    valid_nat = (ts[None, :] < L[:, None]).astype(np.float32)        # [64, T]
    return dict(
        consts,
        logits_il=np.ascontiguousarray(lg_il),
        valid_nat=valid_nat,
    )


def crf_kernel(tc: TileContext, outs, ins, T: int = T, CK: int = 16, CKB: int = 32,
               NP: int = 26, repeat: int = 1):
    nc = tc.nc
    logits_il = ins["logits_il"]      # [128, T, 24] dram f32 (p = 2b+ch interleaved)
    tags_out = outs["tags"]           # [64, T] dram i32

    alpha_dram = nc.dram_tensor("alpha_scratch", [128, T, HALF], F32, kind="Internal").ap()
    a_v = alpha_dram.rearrange("(b h) t c -> b h t c", h=2)

    import contextlib
    with (
        tc.tile_pool(name="persist", bufs=1) as pp,
        tc.tile_pool(name="chunks", bufs=3) as cp,
        tc.tile_pool(name="bchunks", bufs=3) as bp,
        tc.tile_pool(name="work", bufs=4) as wp,
        tc.tile_pool(name="psum", bufs=2, space="PSUM") as xp,
        tc.For_i(0, repeat, 1) if repeat > 1 else contextlib.nullcontext(),
    ):
        # ---- persistent constants ----
        trans_rep = pp.tile([128, HALF, D], F32, tag="trans_rep")
        nc.sync.dma_start(trans_rep[:].rearrange("p a b -> p (a b)"), ins["trans_rep"])
        transT3 = pp.tile([D, 3 * D], BF16, tag="transT3")
        nc.sync.dma_start(transT3[:], ins["transT3"])
        ident64 = pp.tile([BLOC, BLOC], BF16, tag="ident64")
        nc.sync.dma_start(ident64[:], ins["ident64"])
        iota_m64 = pp.tile([BLOC, D], F32, tag="iota_m64")
        nc.sync.dma_start(iota_m64[:], ins["iota_m64"])
        iota_big = pp.tile([BLOC, D], F32, tag="iota_big")
        nc.sync.dma_start(iota_big[:], ins["iota_big"])

        # ---- forward scan: value chain only; alpha history -> DRAM ----
        prev_ref = [None]  # (tile, slot) holding alpha_{t-1}

        for t0 in range(0, T, CK):
            ck = min(CK, T - t0)
            emit_ch = cp.tile([128, CK, HALF], F32, tag="emit_ch")
            nc.sync.dma_start(emit_ch[:, 0:ck, :], logits_il[:, t0:t0 + ck, :])
            ah = cp.tile([128, CK, D], F32, tag="ah")
            for t in range(t0, t0 + ck):
                k = t - t0
                if t == 0:
                    # alpha_0 = logits[:, 0, :]
                    nc.vector.tensor_copy(out=ah[:, 0, 0:HALF], in_=emit_ch[:, 0, :])
                    nc.vector.stream_shuffle(
                        ah[:, 0, HALF:D], ah[:, 0, 0:HALF], mask=PAIR_SWAP_MASK
                    )
                    prev_ref[0] = (ah, 0)
                    continue
                pt, pk = prev_ref[0]
                # fused tropical matvec: one DVE pass computes the running
                # max of trans+alpha within each 48-wide cur segment; the
                # segment max sits at column 47, read strided by the emit add.
                # Padded-step trans zeroing is NOT needed (the backward
                # recomputes candidate rows independently).
                alpha_b = pt[:, pk, :].unsqueeze(1).broadcast_to([128, HALF, D])
                runmax = wp.tile([128, HALF, D], F32, tag="runmax")
                nc.vector._custom_dve(
                    CRF_SEGMAX_ADD, out=runmax[:], in0=trans_rep[:], in1=alpha_b,
                )
                nc.vector.tensor_add(
                    out=ah[:, k, 0:HALF], in0=runmax[:, :, D - 1], in1=emit_ch[:, k, :]
                )
                nc.vector.stream_shuffle(
                    ah[:, k, HALF:D], ah[:, k, 0:HALF], mask=PAIR_SWAP_MASK
                )
                prev_ref[0] = (ah, k)
            nc.sync.dma_start(alpha_dram[:, t0:t0 + ck, :], ah[:, 0:ck, 0:HALF])

        # Merged backward: one pipeline; both b-halves processed by single
        # [64, .] DVE ops (all operands at base partition 0 — custom DVE
        # ops drop nonzero bases on HW). The one-hot transpose is a SINGLE
        # PE op (lhsT = h [64, 48], rhs = ident64 [64, 64] -> [48, 64]
        # PSUM; one tile, no column sharing), one ACT copy, and one
        # 3-matmul exact gather serve all 64 sequences. Minimizes
        # per-instruction overhead, which dominates on HW.
        valid_nat = pp.tile([BLOC, T], F32, tag="valid_nat")
        nc.sync.dma_start(valid_nat[:], ins["valid_nat"])

        # ---- last_tag from final alpha (natural [64, 48]) ----
        alpha_nat = pp.tile([BLOC, D], F32, tag="alpha_nat")
        for hh in range(2):
            nc.sync.dma_start(
                alpha_nat[:, HALF * hh:HALF * (hh + 1)], a_v[:, hh, T - 1, :]
            )
        amax = pp.tile([BLOC, 1], F32, tag="amax")
        nc.vector.tensor_reduce(
            out=amax[:], in_=alpha_nat[:], axis=mybir.AxisListType.X, op=AL.max,
        )
        # mask*(iota-64): negative at argmaxes -> min = first argmax - 64
        fmin0 = pp.tile([BLOC, D], F32, tag="fmin0")
        nc.vector.scalar_tensor_tensor(
            out=fmin0[:], in0=alpha_nat[:], scalar=amax[:, 0:1],
            in1=iota_m64[:], op0=AL.is_ge, op1=AL.mult,
        )
        tagsq = pp.tile([BLOC, T], F32, tag="tagsq")   # tag-64 per t
        nc.vector.tensor_reduce(
            out=tagsq[:, T - 1:T], in_=fmin0[:],
            axis=mybir.AxisListType.X, op=AL.min,
        )

        # ---- backward: recompute one score row per (b, t) ----
        # Per step: P(t): h = onehot(tag_{t+1}) [64,48] bf16; one PE
        # transpose -> [48,64] PSUM; one ACT copy -> SBUF; 3 accumulated
        # bf16 matmuls (exact transT split) -> tsel [64,48] PSUM.
        # D(t): fused cand = tsel*valid + alpha / maxr = max(cand); fused
        # first-argmax (cand >= maxr)*(iota-64) -> min -> tagsq[:, t].
        FMAX = 3.4028234663852886e38

        ach_tiles = {}   # chunk tlo -> tile

        def load_chunk(c0v):
            ckb = min(CKB, c0v + 1)
            tlo = c0v - ckb + 1
            ach = bp.tile([BLOC, CKB, D], F32, tag="ach", name="ach")
            for hh in range(2):
                nc.sync.dma_start(
                    ach[:, 0:ckb, HALF * hh:HALF * (hh + 1)],
                    a_v[:, hh, tlo:tlo + ckb, :],
                )
            ach_tiles[tlo] = ach
            return tlo

        pend = [None]    # tsel_ps tile for the pending P

        def emit_P(t):
            h = wp.tile([BLOC, D], BF16, tag="h", name="h")
            nc.vector.tensor_scalar(
                out=h[:], in0=iota_m64[:], scalar1=tagsq[:, t + 1:t + 2],
                scalar2=None, op0=AL.is_equal,
            )
            hT_ps = xp.tile([D, BLOC], BF16, tag="hT_ps", name="hT_ps")
            nc.tensor.matmul(hT_ps[:], h[:], ident64[:], is_transpose=True)
            hT_sb = wp.tile([D, BLOC], BF16, tag="hT_sb", name="hT_sb")
            nc.scalar.copy(out=hT_sb[:], in_=hT_ps[:])
            tsel_ps = xp.tile([BLOC, D], F32, tag="tsel_ps", name="tsel_ps")
            for kq in range(3):
                nc.tensor.matmul(tsel_ps[:], hT_sb[:],
                                 transT3[:, kq * D:(kq + 1) * D],
                                 start=(kq == 0), stop=(kq == 2))
            pend[0] = tsel_ps

        def emit_D(t, ach, kk):
            cand = wp.tile([BLOC, D], F32, tag="cand", name="cand")
            maxr = wp.tile([BLOC, 1], F32, tag="maxr", name="maxr")
            nc.vector._custom_dve(
                CRF_CAND_MAX, out=cand[:], accum_out=maxr[:],
                in0=pend[0][:],
                in1=ach[:, kk, :], s0=valid_nat[:, t + 1:t + 2],
            )
            fjunk = wp.tile([BLOC, D], F32, tag="fjunk", name="fjunk")
            nc.vector._custom_dve(
                CRF_SELMIN, out=fjunk[:], accum_out=tagsq[:, t:t + 1],
                in0=cand[:], in1=iota_m64[:],
                s0=maxr[:, 0:1], s1=FMAX,
            )

        PREF = 16        # steps of lead time for the next chunk's DMA
        tlo_cur = load_chunk(T - 2)
        tlo_next = None
        emit_P(T - 2)
        for t in range(T - 2, -1, -1):
            if t < tlo_cur:
                tlo_cur = tlo_next
                tlo_next = None
            if tlo_cur > 0 and t == tlo_cur + PREF:
                tlo_next = load_chunk(tlo_cur - 1)
            ach = ach_tiles[tlo_cur]
            kk = t - tlo_cur
            emit_D(t, ach, kk)
            if t > 0:
                emit_P(t - 1)

        if ins.get("_debug_alpha") is not None:
            nc.sync.dma_start(ins["_debug_alpha"], alpha_dram[:])

        # ---- final masking + cast + store ----
        tags_f = pp.tile([BLOC, T], F32, tag="tags_f")
        nc.vector.scalar_tensor_tensor(
            out=tags_f[:], in0=tagsq[:], scalar=BIG,
            in1=valid_nat[:], op0=AL.add, op1=AL.mult,
        )
        tags_i = pp.tile([BLOC, T], I32, tag="tags_i")
        nc.vector.tensor_copy(out=tags_i[:], in_=tags_f[:])
        nc.sync.dma_start(tags_out, tags_i[:])


# ---------------------------------------------------------------------------
# self-contained harness: build once, shard, run SPMD on 8 cores, unshard
# ---------------------------------------------------------------------------
import concourse.bacc as bacc
from concourse.bass_utils import run_bass_kernel_spmd


def _input_specs():
    return {
        "logits_il": ([128, T, HALF], F32),
        "trans_rep": ([128, HALF * D], F32),
        "transT3": ([D, 3 * D], BF16),
        "ident64": ([BLOC, BLOC], BF16),
        "iota_m64": ([BLOC, D], F32),
        "iota_big": ([BLOC, D], F32),
        "valid_nat": ([BLOC, T], F32),
    }


_NC = {}


def _build_nc(repeat: int = 1):
    if repeat in _NC:
        return _NC[repeat]
    nc = bacc.Bacc(
        "TRN2",
        target_bir_lowering=False,
        debug=False,
        enable_asserts=True,
        num_devices=N_CORES,
    )
    ins = {
        name: nc.dram_tensor(name, shape, dt, kind="ExternalInput").ap()
        for name, (shape, dt) in _input_specs().items()
    }
    outs = {"tags": nc.dram_tensor("tags", [BLOC, T], I32, kind="ExternalOutput").ap()}
    with TileContext(nc) as tc:
        crf_kernel(tc, outs, ins, T=T, repeat=repeat)
    nc.compile()
    _NC[repeat] = nc
    return nc


def kernel(logits, sent_lengths, crf_params):
    logits = np.asarray(logits, dtype=np.float32)
    sent_lengths = np.asarray(sent_lengths)
    consts = make_consts(crf_params, T)

    nc = _build_nc()
    in_maps = []
    for core in range(N_CORES):
        lg = logits[core * BLOC:(core + 1) * BLOC]
        sl = sent_lengths[core * BLOC:(core + 1) * BLOC]
        in_maps.append(make_core_inputs(lg, sl, consts))

    br = run_bass_kernel_spmd(nc, in_maps, core_ids=list(range(N_CORES)))
    out = np.concatenate(
        [br.results[core]["tags"] for core in range(N_CORES)], axis=0
    )
    return out.astype(np.int32)



# revision 4
# speedup vs baseline: 2.9924x; 2.9924x over previous
"""nn_CRFLayer: CRF Viterbi decode on 8 Trainium2 NeuronCores.

Data parallel over batch: each core decodes 64 of the 512 sequences.
Self-contained: hardcodes B=512, T=512, D=48, n_cores=8.

Design (v3): the forward pass stores BACKPOINTER CODES in SBUF; the
backward pass is a single tiny DVE gather op per step (no PE, no ACT,
no DRAM round-trip).

  Forward (all DVE, 4 ops/step):
    - CRF_CODES: hand-written 8-stage uop program; one 1152-elem pass
      computes, per 48-wide cur page, the first-tie argmax of
      s = trans*valid + alpha via a swap-flop strict-record compare
      (swap of a MIN stage holds the running max with one-element lag,
      so isnew = [s > m_old] exactly).  write_subdim_last emits one
      1-based code per page -> bphist[:, t, 0:24] directly.
    - CRF_SEG_M: 3-stage segmented max-scan of the same stream;
      write_subdim_last emits the 24 segment maxima (= new alpha
      pre-emit).  Bitwise-identical values to the reference scan.
    - tensor_add m + emit -> ah own half; stream_shuffle rebuilds the
      full 48-wide alpha row across partition pairs.
  Layout: partitions = (b, half) interleaved (p = 2b+ch), 24 cur pages
  x 48 prev per partition, prev rotated per half with pre-rotated trans.
  Padded steps multiply trans by 0 (s0 = valid per partition): alpha
  evolves freely but bp rows at t = L become all-argmax(alpha_{L-1}),
  which re-syncs the backward chain to last_tag.

  Post-forward (3 batch ops): CRF_DECODE converts raw codes to global
  prev-tag ids ((code-1+24h) mod 48) into cols 24:48; one pair-swap
  stream_shuffle copies the partner's decoded half into cols 0:24.
  bphist rows become [partner-tags | own-tags].

  Backward (1 op/step): CRF_GATHER: accum_out = max((iota_pp == tag) *
  bp_row) chases the pointer chain entirely on the DVE, 48 elems/step.

  Output: tags = tagsq * valid, int cast, partition-stride-2 DMA of the
  even partitions -> [64, T].
"""

import numpy as np

import concourse.bass as bass
import concourse.mybir as mybir
from concourse.tile import TileContext


# ---------------------------------------------------------------------------
# custom DVE ops
# ---------------------------------------------------------------------------
def _install_segscan_patches():
    import concourse.dve_spec as dsp
    if getattr(dsp, "_crf_segscan_patched", False):
        return
    _orig_overrides = dsp._scan_overrides

    def _patched(scans, node_stage):
        seed, step = _orig_overrides(scans, node_stage)
        for sc in scans:
            if sc._subdim_step is dsp.MaxNeg:
                step[node_stage[sc]] = dsp._Stage(dsp.AluOp.BYPASS, sc.expr)
            elif sc._subdim_step is dsp.One:
                step[node_stage[sc]] = dsp._Stage(
                    sc.op, dsp._scan_init(sc), sc.expr)
        return seed, step

    dsp._scan_overrides = _patched
    _orig_nas = dsp._node_as_stage

    def _patched_nas(e):
        if isinstance(e, dsp.Scan) and (
            e._subdim_step is dsp.MaxNeg or e._subdim_step is dsp.One
        ):
            from concourse.dve_uop import AluInp as _AI
            return dsp._Stage(e.op, _AI.CURR_ALU_OUT, e.expr)
        return _orig_nas(e)

    dsp._node_as_stage = _patched_nas
    dsp._crf_segscan_patched = True


def _reg(name, spec, subdim=False, patch=None):
    import concourse.dve_ops as dvo
    from concourse.dve_spec import _has_src1, lower as dve_lower
    from concourse.dve_uop import DveOpSpec

    if name in dvo.CUSTOM_DVE_SPECS:
        return next(op for op in dvo.OPS if op.name == name)
    row = dvo._CUSTOM_DVE_ROW_BASE + len(dvo.OPS)
    shas = {}
    specs = {}
    for ver in ("v3", "v4"):
        uops = dve_lower(spec, ver=ver)
        if patch is not None:
            uops = patch(uops, ver)
        s = DveOpSpec(name=name, opcode=row, uops=uops, rd1_en=_has_src1(spec))
        shas[ver] = s.sha(ver)
        specs[ver] = s
    op = dvo.DveOp(name, spec, subdim=subdim, uops_sha=shas)
    for ver in ("v3", "v4"):
        dvo._COMPILE_CACHE[(name, ver)] = specs[ver]
    dvo.OPS.append(op)
    dvo.CUSTOM_DVE_SPECS[name] = spec
    dvo._SUB_OPCODE_FOR_NAME[name] = row
    return op


def _build_codes_uops():
    """Hand-written 8-stage program: first-tie argmax codes per page.

    st0 sv=Src0*C0; st1 s=sv+Src1; st2 M'=min(swap,s) [swap:=running max,
    one-element lag]; st3 isnew=(M'<s); st4 m-scan (unused output, keeps
    structure cheap to reason about); st5 jp counter; st6 cand=isnew*jp;
    st7 a-scan.  Out (subdim-last) = a = 1-based first-tie argmax.
    """
    from concourse.dve_uop import (
        UopConfig, OutPath, OutSel, DelayInp, AluInp, InpSel, Trigger,
        AluOp, ENABLE,
    )
    LANES = [InpSel.SRC_0, InpSel.CONST_0, InpSel.SRC_1, InpSel.ONE_F32,
             InpSel.MAX_NEG, InpSel.ZERO]
    PREV = AluInp.PREV_ALU_OUT
    CURR = AluInp.CURR_ALU_OUT
    D0 = AluInp.PREV_DELAY_0
    D1 = AluInp.PREV_DELAY_1
    D3 = AluInp.PREV_DELAY_3
    D4 = AluInp.PREV_DELAY_4
    D5 = AluInp.PREV_DELAY_5

    def mk(kind):
        u = UopConfig()
        for i, sel in enumerate(LANES):
            u.enable_input(sel, i + 1)
        dp = u.datapath_config
        for st in range(8):
            dp[st].pass_through_delay(0, 1, 2, 3, 4, 5)
        dp[0].enable_alu(AluOp.MULTIPLY, D0, D1)
        dp[1].enable_alu(AluOp.ADD, PREV, D2_ := AluInp.PREV_DELAY_2)
        if kind == "steady":
            dp[2].enable_alu(AluOp.MIN, AluInp.CURR_SWAP_OUT, PREV)
            dp[2].swap_enable = ENABLE
        elif kind == "step":
            dp[2].enable_alu(AluOp.MIN, D4, PREV)
            dp[2].swap_enable = ENABLE
        else:
            dp[2].enable_alu(AluOp.BYPASS, D4, D4)
            dp[2].swap_enable = ENABLE
        dp[2].enable_delay_from_src(DelayInp.PREV_ALU_OUT, 0)  # s -> lane0
        dp[3].enable_alu(AluOp.IS_LT, PREV, D0)
        if kind == "steady":
            dp[4].enable_alu(AluOp.MAX, CURR, D0)
        elif kind == "step":
            dp[4].enable_alu(AluOp.BYPASS, D0, D0)
        else:
            dp[4].enable_alu(AluOp.BYPASS, D4, D4)
        dp[4].enable_delay_from_src(DelayInp.PREV_ALU_OUT, 1)  # isnew -> lane1
        if kind == "steady":
            dp[5].enable_alu(AluOp.ADD, CURR, D3)
        elif kind == "step":
            dp[5].enable_alu(AluOp.BYPASS, D3, D3)
        else:
            dp[5].enable_alu(AluOp.BYPASS, D5, D5)
        dp[6].enable_alu(AluOp.MULTIPLY, D1, PREV)
        if kind == "steady":
            dp[7].enable_alu(AluOp.MAX, CURR, PREV)
        elif kind == "step":
            dp[7].enable_alu(AluOp.BYPASS, PREV, PREV)
        else:
            dp[7].enable_alu(AluOp.BYPASS, D4, D4)
        if kind == "seed":
            u.trigger = (Trigger.COUNT, Trigger.NONE, Trigger.NONE)
            u.next_uop = (1, 0, 0)
            u.repeat_count = 1
        elif kind == "steady":
            u.trigger = (Trigger.SRC_TENSOR_DONE, Trigger.SUB_DIM_DONE,
                         Trigger.NONE)
            u.next_uop = (0, 2, 0)
            u.require_inp0 = u.require_inp1 = ENABLE
        else:
            u.trigger = (Trigger.SRC_TENSOR_DONE, Trigger.SUB_DIM_DONE,
                         Trigger.COUNT)
            u.next_uop = (0, 2, 1)
            u.repeat_count = 1
            u.require_inp0 = u.require_inp1 = ENABLE
        if kind != "seed":
            u.enable_output(OutSel.ALU_OUT, OutPath.WR0_LO)
            u.out_last_subdim_enable = ENABLE
        return u

    return [mk("seed"), mk("steady"), mk("step")]


def _register_ops():
    import concourse.dve_spec as dsp
    from concourse.dve_spec import (
        Spec, Src0, Src1, C0, C1, maxx, AluOp, Bin, MaxNeg, Scan,
    )
    _install_segscan_patches()

    # ---- CRF_CODES: hand uops; spec only provides rd1_en/leaves/ref ----
    def _ref_codes(in0, in1, c0, c1, c2):
        s = (in0 * c0 + in1).astype(np.float32)
        code = s.argmax(axis=-1) + 1.0
        out = np.zeros_like(s)
        out[..., -1] = code
        return out

    codes_uops = _build_codes_uops()
    codes = _reg("CRF_CODES", Spec(
        body=Bin(AluOp.ADD, Bin(AluOp.MULTIPLY, Src0, C0), Src1),
        reference=_ref_codes,
    ), subdim=True, patch=lambda u, ver: codes_uops)

    # ---- CRF_SEG_M: segmented max-scan, subdim-last out ----
    def _ref_m(in0, in1, c0, c1, c2):
        s = (in0 * c0 + in1).astype(np.float32)
        return np.maximum.accumulate(s, axis=-1)

    def patch_subdim_last(uops, ver):
        import copy
        from concourse.dve_uop import ENABLE
        uops = copy.deepcopy(uops)
        for u in uops[1:]:
            u.out_last_subdim_enable = ENABLE
        return uops

    seg_m = _reg("CRF_SEG_M", Spec(
        body=Scan(AluOp.MAX, Bin(AluOp.ADD, Bin(AluOp.MULTIPLY, Src0, C0),
                                 Src1), _subdim_step=MaxNeg),
        reference=_ref_m,
    ), subdim=True, patch=patch_subdim_last)

    # ---- CRF_GATHER: accum_out = max((Src0==C0)*Src1) ----
    def _ref_gather(in0, in1, c0, c1, c2):
        b = ((in0 == c0) * in1).astype(np.float32)
        mx = b.reshape(b.shape[0], -1).max(axis=-1, keepdims=True)
        return b, np.maximum(mx, np.float32(-3.4e38))

    gather = _reg("CRF_GATHER", Spec(
        body=Bin(AluOp.MULTIPLY, Bin(AluOp.IS_EQ, Src0, C0), Src1),
        accum=maxx, reference=_ref_gather,
    ))

    # ---- CRF_DECODE: (Src0 + C0) - ((Src0 + C0) >= C1)*C1 ----
    def _ref_decode(in0, in1, c0, c1, c2):
        u = in0 + c0
        return (u - (u >= c1) * c1).astype(np.float32)

    u_expr = Bin(AluOp.ADD, Src0, C0)
    decode = _reg("CRF_DECODE", Spec(
        body=Bin(AluOp.SUBTRACT, u_expr,
                 Bin(AluOp.MULTIPLY, Bin(AluOp.IS_GE, u_expr, C1), C1)),
        reference=_ref_decode,
    ))
    return codes, seg_m, gather, decode


CRF_CODES, CRF_SEG_M, CRF_GATHER, CRF_DECODE = _register_ops()

AL = mybir.AluOpType
F32 = mybir.dt.float32
I32 = mybir.dt.int32

D = 48
HALF = 24
BLOC = 64
B = 512
T = 512
N_CORES = 8

PAIR_SWAP_MASK = [i ^ 1 for i in range(32)]


# ---------------------------------------------------------------------------
# host-side constant prep
# ---------------------------------------------------------------------------
def make_consts(trans: np.ndarray, T: int = T) -> dict[str, np.ndarray]:
    trans = np.asarray(trans, dtype=np.float32)
    # trans_rep[p=2b+ch] = [cur24 pages, prev48 rotated by 24*ch]
    trans_rep = np.zeros((128, HALF, D), dtype=np.float32)
    for ch in range(2):
        prev = (np.arange(D) + HALF * ch) % D
        cur = HALF * ch + np.arange(HALF)
        block = trans[prev][:, cur].T  # [cur24, prev48]
        for b in range(BLOC):
            trans_rep[2 * b + ch] = block
    # iota_pp[p, q]: tag ids of bp_full rows laid out [partner | own]
    iota_pp = np.zeros((128, D), dtype=np.float32)
    # iota_ah[p, q]: tag ids of ah rows laid out [own | partner]
    iota_ah = np.zeros((128, D), dtype=np.float32)
    for ch in range(2):
        own = HALF * ch + np.arange(HALF)
        par = HALF * (1 - ch) + np.arange(HALF)
        iota_pp[ch::2, 0:HALF] = par
        iota_pp[ch::2, HALF:D] = own
        iota_ah[ch::2, 0:HALF] = own
        iota_ah[ch::2, HALF:D] = par
    # decode offset per partition: c0 = 24*ch - 1
    dec_c0 = np.zeros((128, 1), dtype=np.float32)
    dec_c0[0::2, 0] = -1.0
    dec_c0[1::2, 0] = HALF - 1.0
    return {
        "trans_rep": trans_rep.reshape(128, HALF * D),
        "iota_pp": iota_pp,
        "iota_ah_m64": iota_ah - 64.0,
        "dec_c0": dec_c0,
    }


def make_core_inputs(logits_core, sent_lengths_core, consts):
    L = np.asarray(sent_lengths_core, dtype=np.float32)
    lg = np.asarray(logits_core, dtype=np.float32)
    Tv = lg.shape[1]
    lg_il = lg.reshape(BLOC, Tv, 2, HALF).transpose(0, 2, 1, 3).reshape(
        128, Tv, HALF)
    ts = np.arange(Tv, dtype=np.float32)
    valid_nat = (ts[None, :] < L[:, None]).astype(np.float32)   # [64, T]
    valid_il = np.repeat(valid_nat, 2, axis=0)                  # [128, T]
    return dict(
        consts,
        logits_il=np.ascontiguousarray(lg_il),
        valid_il=np.ascontiguousarray(valid_il),
    )


# ---------------------------------------------------------------------------
# the kernel
# ---------------------------------------------------------------------------
def crf_kernel(tc: TileContext, outs, ins, T: int = T, CK: int = 32,
               repeat: int = 1):
    nc = tc.nc
    logits_il = ins["logits_il"]      # [128, T, 24] dram f32
    tags_out = outs["tags"]           # [64, T] dram i32

    import contextlib
    with (
        tc.tile_pool(name="persist", bufs=1) as pp,
        tc.tile_pool(name="chunks", bufs=3) as cp,
        tc.tile_pool(name="work", bufs=2) as wp,
        tc.For_i(0, repeat, 1) if repeat > 1 else contextlib.nullcontext(),
    ):
        # ---- persistent constants / state ----
        trans_rep = pp.tile([128, HALF, D], F32, tag="trans_rep")
        nc.sync.dma_start(trans_rep[:].rearrange("p a b -> p (a b)"),
                          ins["trans_rep"])
        iota_pp = pp.tile([128, D], F32, tag="iota_pp")
        nc.sync.dma_start(iota_pp[:], ins["iota_pp"])
        iota_ah_m64 = pp.tile([128, D], F32, tag="iota_ah_m64")
        nc.sync.dma_start(iota_ah_m64[:], ins["iota_ah_m64"])
        dec_c0 = pp.tile([128, 1], F32, tag="dec_c0")
        nc.sync.dma_start(dec_c0[:], ins["dec_c0"])
        valid_il = pp.tile([128, T], F32, tag="valid_il")
        nc.sync.dma_start(valid_il[:], ins["valid_il"])

        bphist = pp.tile([128, T, D], F32, tag="bphist")   # codes then tags
        ah = [pp.tile([128, D], F32, tag=f"ah{i}", name=f"ah{i}") for i in range(2)]
        mrow = [pp.tile([128, HALF], F32, tag=f"mrow{i}", name=f"mrow{i}") for i in range(2)]

        # ---- forward scan ----
        for t0 in range(0, T, CK):
            ck = min(CK, T - t0)
            emit_ch = cp.tile([128, CK, HALF], F32, tag="emit_ch")
            nc.sync.dma_start(emit_ch[:, 0:ck, :], logits_il[:, t0:t0 + ck, :])
            for t in range(t0, t0 + ck):
                k = t - t0
                cur, prv = ah[t % 2], ah[(t + 1) % 2]
                if t == 0:
                    nc.vector.tensor_copy(out=cur[:, 0:HALF],
                                          in_=emit_ch[:, 0, :])
                    nc.vector.stream_shuffle(cur[:, HALF:D], cur[:, 0:HALF],
                                             mask=PAIR_SWAP_MASK)
                    continue
                alpha_b = prv[:].unsqueeze(1).broadcast_to([128, HALF, D])
                # backpointer codes straight into bphist
                nc.vector._custom_dve(
                    CRF_CODES, out=bphist[:, t, 0:HALF], in0=trans_rep[:],
                    in1=alpha_b, s0=valid_il[:, t:t + 1],
                )
                # segment maxima (new alpha pre-emit)
                mr = mrow[t % 2]
                nc.vector._custom_dve(
                    CRF_SEG_M, out=mr[:], in0=trans_rep[:],
                    in1=alpha_b, s0=valid_il[:, t:t + 1],
                )
                nc.vector.tensor_add(out=cur[:, 0:HALF], in0=mr[:],
                                     in1=emit_ch[:, k, :])
                nc.vector.stream_shuffle(cur[:, HALF:D], cur[:, 0:HALF],
                                         mask=PAIR_SWAP_MASK)

        ah_fin = ah[(T - 1) % 2]

        # ---- decode codes -> global tags; shuffle partner half ----
        nc.vector._custom_dve(
            CRF_DECODE, out=bphist[:, 1:T, HALF:D],
            in0=bphist[:, 1:T, 0:HALF],
            s0=dec_c0[:, 0:1], s1=48.0,
        )
        nc.vector.stream_shuffle(
            bphist[:, 1:T, 0:HALF], bphist[:, 1:T, HALF:D],
            mask=PAIR_SWAP_MASK,
        )

        # ---- last_tag: first-tie argmax of final alpha ----
        tagsq = pp.tile([128, T], F32, tag="tagsq")
        amax = wp.tile([128, 1], F32, tag="amax")
        nc.vector.tensor_reduce(out=amax[:], in_=ah_fin[:],
                                axis=mybir.AxisListType.X, op=AL.max)
        fmin0 = wp.tile([128, D], F32, tag="fmin0")
        nc.vector.scalar_tensor_tensor(
            out=fmin0[:], in0=ah_fin[:], scalar=amax[:, 0:1],
            in1=iota_ah_m64[:], op0=AL.is_ge, op1=AL.mult,
        )
        nc.vector.tensor_reduce(out=tagsq[:, T - 1:T], in_=fmin0[:],
                                axis=mybir.AxisListType.X, op=AL.min)
        nc.vector.tensor_scalar(
            out=tagsq[:, T - 1:T], in0=tagsq[:, T - 1:T], scalar1=64.0,
            scalar2=None, op0=AL.add,
        )

        # ---- backward: one gather per step ----
        junk = pp.tile([128, D], F32, tag="junk")
        for t in range(T - 2, -1, -1):
            nc.vector._custom_dve(
                CRF_GATHER, out=junk[:], accum_out=tagsq[:, t:t + 1],
                in0=iota_pp[:], in1=bphist[:, t + 1, :],
                s0=tagsq[:, t + 1:t + 2],
            )

        # ---- mask + cast + store (even partitions) ----
        tags_f = pp.tile([128, T], F32, tag="tags_f")
        nc.vector.tensor_mul(out=tags_f[:], in0=tagsq[:], in1=valid_il[:])
        tags_i = pp.tile([128, T], I32, tag="tags_i")
        nc.vector.tensor_copy(out=tags_i[:], in_=tags_f[:])
        ev = tags_i[:].rearrange("(b c) t -> b c t", c=2)
        nc.sync.dma_start(tags_out, ev[:, 0, :])


# ---------------------------------------------------------------------------
# self-contained harness: build once, shard, run SPMD on 8 cores, unshard
# ---------------------------------------------------------------------------
import concourse.bacc as bacc
from concourse.bass_utils import run_bass_kernel_spmd


def _input_specs():
    return {
        "logits_il": ([128, T, HALF], F32),
        "trans_rep": ([128, HALF * D], F32),
        "iota_pp": ([128, D], F32),
        "iota_ah_m64": ([128, D], F32),
        "dec_c0": ([128, 1], F32),
        "valid_il": ([128, T], F32),
    }


_NC = {}


def _build_nc(repeat: int = 1):
    if repeat in _NC:
        return _NC[repeat]
    nc = bacc.Bacc(
        "TRN2",
        target_bir_lowering=False,
        debug=False,
        enable_asserts=True,
        num_devices=N_CORES,
    )
    ins = {
        name: nc.dram_tensor(name, shape, dt, kind="ExternalInput").ap()
        for name, (shape, dt) in _input_specs().items()
    }
    outs = {"tags": nc.dram_tensor("tags", [BLOC, T], I32,
                                   kind="ExternalOutput").ap()}
    with TileContext(nc) as tc:
        crf_kernel(tc, outs, ins, T=T, repeat=repeat)
    nc.compile()
    _NC[repeat] = nc
    return nc


def kernel(logits, sent_lengths, crf_params):
    logits = np.asarray(logits, dtype=np.float32)
    sent_lengths = np.asarray(sent_lengths)
    consts = make_consts(crf_params, T)

    nc = _build_nc()
    in_maps = []
    for core in range(N_CORES):
        lg = logits[core * BLOC:(core + 1) * BLOC]
        sl = sent_lengths[core * BLOC:(core + 1) * BLOC]
        in_maps.append(make_core_inputs(lg, sl, consts))

    br = run_bass_kernel_spmd(nc, in_maps, core_ids=list(range(N_CORES)))
    out = np.concatenate(
        [br.results[core]["tags"] for core in range(N_CORES)], axis=0
    )
    return out.astype(np.int32)


# revision 10
# speedup vs baseline: 5.1706x; 1.7279x over previous
"""nn_CRFLayer: CRF Viterbi decode on 8 Trainium2 NeuronCores.

Data parallel over batch: each core decodes 64 of the 512 sequences.
Self-contained: hardcodes B=512, T=512, D=48, n_cores=8.

Design (v3): the forward pass stores BACKPOINTER CODES in SBUF; the
backward pass is a single tiny DVE gather op per step (no PE, no ACT,
no DRAM round-trip).

  Forward (all DVE, 4 ops/step):
    - CRF_CODES: hand-written 8-stage uop program; one 1152-elem pass
      computes, per 48-wide cur page, the first-tie argmax of
      s = trans*valid + alpha via a swap-flop strict-record compare
      (swap of a MIN stage holds the running max with one-element lag,
      so isnew = [s > m_old] exactly).  write_subdim_last emits one
      1-based code per page -> bphist[:, t, 0:24] directly.
    - CRF_SEG_M: 3-stage segmented max-scan of the same stream;
      write_subdim_last emits the 24 segment maxima (= new alpha
      pre-emit).  Bitwise-identical values to the reference scan.
    - tensor_add m + emit -> ah own half; stream_shuffle rebuilds the
      full 48-wide alpha row across partition pairs.
  Layout: partitions = (b, half) interleaved (p = 2b+ch), 24 cur pages
  x 48 prev per partition, prev rotated per half with pre-rotated trans.
  Padded steps multiply trans by 0 (s0 = valid per partition): alpha
  evolves freely but bp rows at t = L become all-argmax(alpha_{L-1}),
  which re-syncs the backward chain to last_tag.

  Post-forward (3 batch ops): CRF_DECODE converts raw codes to global
  prev-tag ids ((code-1+24h) mod 48) into cols 24:48; one pair-swap
  stream_shuffle copies the partner's decoded half into cols 0:24.
  bphist rows become [partner-tags | own-tags].

  Backward (1 op/step): CRF_GATHER: accum_out = max((iota_pp == tag) *
  bp_row) chases the pointer chain entirely on the DVE, 48 elems/step.

  Output: tags = tagsq * valid, int cast, partition-stride-2 DMA of the
  even partitions -> [64, T].
"""

import numpy as np

import concourse.bass as bass
import concourse.mybir as mybir
from concourse.tile import TileContext


# ---------------------------------------------------------------------------
# custom DVE ops
# ---------------------------------------------------------------------------
def _install_segscan_patches():
    import concourse.dve_spec as dsp
    if getattr(dsp, "_crf_segscan_patched", False):
        return
    _orig_overrides = dsp._scan_overrides

    def _patched(scans, node_stage):
        seed, step = _orig_overrides(scans, node_stage)
        for sc in scans:
            if sc._subdim_step is dsp.MaxNeg:
                step[node_stage[sc]] = dsp._Stage(dsp.AluOp.BYPASS, sc.expr)
            elif sc._subdim_step is dsp.One:
                step[node_stage[sc]] = dsp._Stage(
                    sc.op, dsp._scan_init(sc), sc.expr)
        return seed, step

    dsp._scan_overrides = _patched
    _orig_nas = dsp._node_as_stage

    def _patched_nas(e):
        if isinstance(e, dsp.Scan) and (
            e._subdim_step is dsp.MaxNeg or e._subdim_step is dsp.One
        ):
            from concourse.dve_uop import AluInp as _AI
            return dsp._Stage(e.op, _AI.CURR_ALU_OUT, e.expr)
        return _orig_nas(e)

    dsp._node_as_stage = _patched_nas
    dsp._crf_segscan_patched = True


def _reg(name, spec, subdim=False, patch=None):
    import concourse.dve_ops as dvo
    from concourse.dve_spec import _has_src1, lower as dve_lower
    from concourse.dve_uop import DveOpSpec

    if name in dvo.CUSTOM_DVE_SPECS:
        return next(op for op in dvo.OPS if op.name == name)
    row = dvo._CUSTOM_DVE_ROW_BASE + len(dvo.OPS)
    shas = {}
    specs = {}
    for ver in ("v3", "v4"):
        uops = dve_lower(spec, ver=ver)
        if patch is not None:
            uops = patch(uops, ver)
        s = DveOpSpec(name=name, opcode=row, uops=uops, rd1_en=_has_src1(spec))
        shas[ver] = s.sha(ver)
        specs[ver] = s
    op = dvo.DveOp(name, spec, subdim=subdim, uops_sha=shas)
    for ver in ("v3", "v4"):
        dvo._COMPILE_CACHE[(name, ver)] = specs[ver]
    dvo.OPS.append(op)
    dvo.CUSTOM_DVE_SPECS[name] = spec
    dvo._SUB_OPCODE_FOR_NAME[name] = row
    return op


def _build_codes_uops():
    """Hand-written 5-uop program: per 49-elem page (48 real + 1 pad with
    s=-BIG), element 47 writes the first-tie argmax code (a-scan), element
    48 writes the segment max (m via delay lane 2).

    st0 sv=Src0*C0; st1 s=sv+Src1; st2 M'=min(swap,s) [swap:=running max,
    one-element lag]; st3 isnew=(M'<s); st4 m-scan; st5 jp counter;
    st6 cand=isnew*jp; st7 a-scan.
    """
    from concourse.dve_uop import (
        UopConfig, OutPath, OutSel, DelayInp, AluInp, InpSel, Trigger,
        AluOp, ENABLE, DELAY_OUT,
    )
    LANES = [InpSel.SRC_0, InpSel.CONST_0, InpSel.SRC_1, InpSel.ONE_F32,
             InpSel.MAX_NEG, InpSel.ZERO]
    PREV = AluInp.PREV_ALU_OUT
    CURR = AluInp.CURR_ALU_OUT
    D0 = AluInp.PREV_DELAY_0
    D1 = AluInp.PREV_DELAY_1
    D2 = AluInp.PREV_DELAY_2
    D3 = AluInp.PREV_DELAY_3
    D4 = AluInp.PREV_DELAY_4
    D5 = AluInp.PREV_DELAY_5

    def mk(kind):
        u = UopConfig()
        for i, sel in enumerate(LANES):
            u.enable_input(sel, i + 1)
        dp = u.datapath_config
        for st in range(8):
            dp[st].pass_through_delay(0, 1, 2, 3, 4, 5)
        step = kind == "step"
        seed = kind == "seed"
        dp[0].enable_alu(AluOp.MULTIPLY, D0, D1)
        dp[1].enable_alu(AluOp.ADD, PREV, D2)
        if seed:
            dp[2].enable_alu(AluOp.BYPASS, D4, D4)
        elif step:
            dp[2].enable_alu(AluOp.MIN, D4, PREV)
        else:
            dp[2].enable_alu(AluOp.MIN, AluInp.CURR_SWAP_OUT, PREV)
        dp[2].swap_enable = ENABLE
        dp[2].enable_delay_from_src(DelayInp.PREV_ALU_OUT, 0)   # s -> lane0
        dp[3].enable_alu(AluOp.IS_LT, PREV, D0)
        if seed:
            dp[4].enable_alu(AluOp.BYPASS, D4, D4)
        elif step:
            dp[4].enable_alu(AluOp.BYPASS, D0, D0)
        else:
            dp[4].enable_alu(AluOp.MAX, CURR, D0)
        dp[4].enable_delay_from_src(DelayInp.PREV_ALU_OUT, 1)   # isnew -> lane1
        if seed:
            dp[5].enable_alu(AluOp.BYPASS, D5, D5)
        elif step:
            dp[5].enable_alu(AluOp.BYPASS, D3, D3)
        else:
            dp[5].enable_alu(AluOp.ADD, CURR, D3)
        dp[5].enable_delay_from_src(DelayInp.PREV_ALU_OUT, 2)   # m -> lane2
        dp[6].enable_alu(AluOp.MULTIPLY, D1, PREV)
        if seed:
            dp[7].enable_alu(AluOp.BYPASS, D4, D4)
        elif step:
            dp[7].enable_alu(AluOp.BYPASS, PREV, PREV)
        else:
            dp[7].enable_alu(AluOp.MAX, CURR, PREV)

        SRC = Trigger.SRC_TENSOR_DONE
        SUB = Trigger.SUB_DIM_DONE
        CNT = Trigger.COUNT
        NONE = Trigger.NONE
        if kind == "seed":
            u.trigger = (CNT, NONE, NONE)
            u.next_uop = (1, 0, 0)
            u.repeat_count = 1
        elif kind == "step":
            u.trigger = (SRC, SUB, CNT)
            u.next_uop = (0, 1, 2)
            u.repeat_count = 1
            u.require_inp0 = u.require_inp1 = ENABLE
        elif kind == "steady47":
            u.trigger = (SRC, SUB, CNT)
            u.next_uop = (0, 1, 3)
            u.repeat_count = 46
            u.require_inp0 = u.require_inp1 = ENABLE
        elif kind == "wr_code":
            u.trigger = (SRC, SUB, CNT)
            u.next_uop = (0, 1, 4)
            u.repeat_count = 1
            u.require_inp0 = u.require_inp1 = ENABLE
            u.enable_output(OutSel.ALU_OUT, OutPath.WR0_LO)
        else:  # wr_m
            u.trigger = (SRC, SUB, NONE)
            u.next_uop = (0, 1, 0)
            u.require_inp0 = u.require_inp1 = ENABLE
            u.enable_output(DELAY_OUT[2], OutPath.WR0_LO)
        return u

    return [mk("seed"), mk("step"), mk("steady47"), mk("wr_code"),
            mk("wr_m")]


def _register_ops():
    import concourse.dve_spec as dsp
    from concourse.dve_spec import (
        Spec, Src0, Src1, C0, C1, maxx, AluOp, Bin, MaxNeg, Scan,
    )
    _install_segscan_patches()

    # ---- CRF_FUSED_CM: hand uops; spec only provides rd1_en/leaves ----
    def _ref_fused(in0, in1, c0, c1, c2):
        return (in0 * c0 + in1).astype(np.float32)

    fused_uops = _build_codes_uops()
    fused = _reg("CRF_FUSED_CM", Spec(
        body=Bin(AluOp.ADD, Bin(AluOp.MULTIPLY, Src0, C0), Src1),
        reference=_ref_fused,
    ), subdim=True, patch=lambda u, ver: fused_uops)

    # ---- CRF_GATHER: accum_out = max((Src0==C0)*Src1) ----
    def _ref_gather(in0, in1, c0, c1, c2):
        b = ((in0 == c0) * in1).astype(np.float32)
        mx = b.reshape(b.shape[0], -1).max(axis=-1, keepdims=True)
        return b, np.maximum(mx, np.float32(-3.4e38))

    gather = _reg("CRF_GATHER", Spec(
        body=Bin(AluOp.MULTIPLY, Bin(AluOp.IS_EQ, Src0, C0), Src1),
        accum=maxx, reference=_ref_gather,
    ))

    # ---- CRF_DECODE: (Src0 + C0) - ((Src0 + C0) >= C1)*C1 ----
    def _ref_decode(in0, in1, c0, c1, c2):
        u = in0 + c0
        return (u - (u >= c1) * c1).astype(np.float32)

    u_expr = Bin(AluOp.ADD, Src0, C0)
    decode = _reg("CRF_DECODE", Spec(
        body=Bin(AluOp.SUBTRACT, u_expr,
                 Bin(AluOp.MULTIPLY, Bin(AluOp.IS_GE, u_expr, C1), C1)),
        reference=_ref_decode,
    ))
    return fused, gather, decode


CRF_FUSED_CM, CRF_GATHER, CRF_DECODE = _register_ops()

AL = mybir.AluOpType
F32 = mybir.dt.float32
I32 = mybir.dt.int32

D = 48
HALF = 24
BLOC = 64
B = 512
T = 512
N_CORES = 8

PAIR_SWAP_MASK = [i ^ 1 for i in range(32)]


# ---------------------------------------------------------------------------
# host-side constant prep
# ---------------------------------------------------------------------------
def make_consts(trans: np.ndarray, T: int = T) -> dict[str, np.ndarray]:
    trans = np.asarray(trans, dtype=np.float32)
    # trans_rep[p=2b+ch] = [cur24 pages, prev48 rotated by 24*ch, + pad col]
    trans_rep = np.zeros((128, HALF, D + 1), dtype=np.float32)
    for ch in range(2):
        prev = (np.arange(D) + HALF * ch) % D
        cur = HALF * ch + np.arange(HALF)
        block = trans[prev][:, cur].T  # [cur24, prev48]
        for b in range(BLOC):
            trans_rep[2 * b + ch, :, 0:D] = block
    # iota_pp[p]: tag ids of decoded bp rows, interleaved
    #   even col 2q = partner tag, odd col 2q+1 = own tag
    iota_pp = np.zeros((128, D), dtype=np.float32)
    # iota_ah[p, q]: tag ids of ah rows laid out [own | partner]
    iota_ah = np.zeros((128, D), dtype=np.float32)
    for ch in range(2):
        own = HALF * ch + np.arange(HALF)
        par = HALF * (1 - ch) + np.arange(HALF)
        iota_pp[ch::2, 0:D:2] = par
        iota_pp[ch::2, 1:D:2] = own
        iota_ah[ch::2, 0:HALF] = own
        iota_ah[ch::2, HALF:D] = par
    # decode offset per partition: c0 = 24*ch - 1
    dec_c0 = np.zeros((128, 1), dtype=np.float32)
    dec_c0[0::2, 0] = -1.0
    dec_c0[1::2, 0] = HALF - 1.0
    neg_big = np.full((128, 1), -1e30, dtype=np.float32)
    return {
        "trans_rep": trans_rep.reshape(128, HALF * (D + 1)),
        "iota_pp": iota_pp,
        "iota_ah_m64": iota_ah - 64.0,
        "dec_c0": dec_c0,
        "neg_big": neg_big,
    }


def make_core_inputs(logits_core, sent_lengths_core, consts):
    L = np.asarray(sent_lengths_core, dtype=np.float32)
    lg = np.asarray(logits_core, dtype=np.float32)
    Tv = lg.shape[1]
    lg_il = lg.reshape(BLOC, Tv, 2, HALF).transpose(0, 2, 1, 3).reshape(
        128, Tv, HALF)
    ts = np.arange(Tv, dtype=np.float32)
    valid_nat = (ts[None, :] < L[:, None]).astype(np.float32)   # [64, T]
    valid_il = np.repeat(valid_nat, 2, axis=0)                  # [128, T]
    return dict(
        consts,
        logits_il=np.ascontiguousarray(lg_il),
        valid_il=np.ascontiguousarray(valid_il),
    )


# ---------------------------------------------------------------------------
# the kernel
# ---------------------------------------------------------------------------
def crf_kernel(tc: TileContext, outs, ins, T: int = T, CK: int = 32,
               repeat: int = 1, mode: str = "full"):
    nc = tc.nc
    logits_il = ins["logits_il"]      # [128, T, 24] dram f32
    tags_out = outs["tags"]           # [64, T] dram i32

    import contextlib
    with (
        tc.tile_pool(name="persist", bufs=1) as pp,
        tc.tile_pool(name="chunks", bufs=3) as cp,
        tc.tile_pool(name="work", bufs=2) as wp,
        tc.For_i(0, repeat, 1) if repeat > 1 else contextlib.nullcontext(),
    ):
        # ---- persistent constants / state ----
        trans_rep = pp.tile([128, HALF, D + 1], F32, tag="trans_rep")
        nc.sync.dma_start(trans_rep[:].rearrange("p a b -> p (a b)"),
                          ins["trans_rep"])
        iota_pp = pp.tile([128, D], F32, tag="iota_pp")
        nc.sync.dma_start(iota_pp[:], ins["iota_pp"])
        iota_ah_m64 = pp.tile([128, D], F32, tag="iota_ah_m64")
        nc.sync.dma_start(iota_ah_m64[:], ins["iota_ah_m64"])
        dec_c0 = pp.tile([128, 1], F32, tag="dec_c0")
        nc.sync.dma_start(dec_c0[:], ins["dec_c0"])
        valid_il = pp.tile([128, T], F32, tag="valid_il")
        nc.sync.dma_start(valid_il[:], ins["valid_il"])

        bphist = pp.tile([128, T, D], F32, tag="bphist")   # (code,m) pairs
        ah = [pp.tile([128, D + 1], F32, tag=f"ah{i}", name=f"ah{i}")
              for i in range(2)]
        for i in range(2):
            nc.sync.dma_start(ah[i][:, D:D + 1], ins["neg_big"])

        # ---- forward scan ----
        for t0 in range(0, T, CK):
            ck = min(CK, T - t0)
            emit_ch = cp.tile([128, CK, HALF], F32, tag="emit_ch")
            nc.sync.dma_start(emit_ch[:, 0:ck, :], logits_il[:, t0:t0 + ck, :])
            for t in range(t0, t0 + ck):
                k = t - t0
                cur, prv = ah[t % 2], ah[(t + 1) % 2]
                if mode == "fwd1":
                    prv = ah[0]
                if t == 0:
                    nc.vector.tensor_copy(out=cur[:, 0:HALF],
                                          in_=emit_ch[:, 0, :])
                    nc.vector.stream_shuffle(cur[:, HALF:D], cur[:, 0:HALF],
                                             mask=PAIR_SWAP_MASK)
                    continue
                alpha_b = prv[:].unsqueeze(1).broadcast_to(
                    [128, HALF, D + 1])
                # (code, m) pairs straight into bphist
                nc.vector._custom_dve(
                    CRF_FUSED_CM, out=bphist[:, t, :], in0=trans_rep[:],
                    in1=alpha_b, s0=valid_il[:, t:t + 1],
                )
                if mode == "fwd1":
                    continue
                nc.vector.tensor_add(out=cur[:, 0:HALF],
                                     in0=bphist[:, t, 1:D:2],
                                     in1=emit_ch[:, k, :])
                nc.vector.stream_shuffle(cur[:, HALF:D], cur[:, 0:HALF],
                                         mask=PAIR_SWAP_MASK)

        ah_fin = ah[0] if mode == "fwd1" else ah[(T - 1) % 2]

        skip_bwd = mode in ("fwd1", "fwd2", "fwd")
        # ---- decode codes -> global tags; shuffle partner half ----
        if not skip_bwd:
            # decode raw codes (even cols) -> own tags onto odd cols
            nc.vector._custom_dve(
                CRF_DECODE, out=bphist[:, 1:T, 1:D:2],
                in0=bphist[:, 1:T, 0:D:2],
                s0=dec_c0[:, 0:1], s1=48.0,
            )
            # partner's decoded tags onto even cols
            nc.vector.stream_shuffle(
                bphist[:, 1:T, 0:D:2], bphist[:, 1:T, 1:D:2],
                mask=PAIR_SWAP_MASK,
            )

        # ---- last_tag: first-tie argmax of final alpha ----
        tagsq = pp.tile([128, T], F32, tag="tagsq")
        amax = wp.tile([128, 1], F32, tag="amax")
        nc.vector.tensor_reduce(out=amax[:], in_=ah_fin[:, 0:D],
                                axis=mybir.AxisListType.X, op=AL.max)
        fmin0 = wp.tile([128, D], F32, tag="fmin0")
        nc.vector.scalar_tensor_tensor(
            out=fmin0[:], in0=ah_fin[:, 0:D], scalar=amax[:, 0:1],
            in1=iota_ah_m64[:], op0=AL.is_ge, op1=AL.mult,
        )
        nc.vector.tensor_reduce(out=tagsq[:, T - 1:T], in_=fmin0[:],
                                axis=mybir.AxisListType.X, op=AL.min)
        nc.vector.tensor_scalar(
            out=tagsq[:, T - 1:T], in0=tagsq[:, T - 1:T], scalar1=64.0,
            scalar2=None, op0=AL.add,
        )

        # ---- backward: one gather per step ----
        junk = pp.tile([128, D], F32, tag="junk")
        if not skip_bwd:
            for t in range(T - 2, -1, -1):
                nc.vector._custom_dve(
                    CRF_GATHER, out=junk[:], accum_out=tagsq[:, t:t + 1],
                    in0=iota_pp[:], in1=bphist[:, t + 1, :],
                    s0=tagsq[:, t + 1:t + 2],
                )

        # ---- mask + cast + store (even partitions) ----
        tags_f = pp.tile([128, T], F32, tag="tags_f")
        nc.vector.tensor_mul(out=tags_f[:], in0=tagsq[:], in1=valid_il[:])
        tags_i = pp.tile([128, T], I32, tag="tags_i")
        nc.vector.tensor_copy(out=tags_i[:], in_=tags_f[:])
        ev = tags_i[:].rearrange("(b c) t -> b c t", c=2)
        nc.sync.dma_start(tags_out, ev[:, 0, :])


# ---------------------------------------------------------------------------
# self-contained harness: build once, shard, run SPMD on 8 cores, unshard
# ---------------------------------------------------------------------------
import concourse.bacc as bacc
from concourse.bass_utils import run_bass_kernel_spmd


def _input_specs():
    return {
        "logits_il": ([128, T, HALF], F32),
        "trans_rep": ([128, HALF * (D + 1)], F32),
        "iota_pp": ([128, D], F32),
        "iota_ah_m64": ([128, D], F32),
        "dec_c0": ([128, 1], F32),
        "neg_big": ([128, 1], F32),
        "valid_il": ([128, T], F32),
    }


_NC = {}


def _build_nc(repeat: int = 1, mode: str = "full"):
    if (repeat, mode) in _NC:
        return _NC[(repeat, mode)]
    nc = bacc.Bacc(
        "TRN2",
        target_bir_lowering=False,
        debug=False,
        enable_asserts=True,
        num_devices=N_CORES,
    )
    ins = {
        name: nc.dram_tensor(name, shape, dt, kind="ExternalInput").ap()
        for name, (shape, dt) in _input_specs().items()
    }
    outs = {"tags": nc.dram_tensor("tags", [BLOC, T], I32,
                                   kind="ExternalOutput").ap()}
    with TileContext(nc) as tc:
        crf_kernel(tc, outs, ins, T=T, repeat=repeat, mode=mode)
    nc.compile()
    _NC[(repeat, mode)] = nc
    return nc


def kernel(logits, sent_lengths, crf_params):
    logits = np.asarray(logits, dtype=np.float32)
    sent_lengths = np.asarray(sent_lengths)
    consts = make_consts(crf_params, T)

    nc = _build_nc()
    in_maps = []
    for core in range(N_CORES):
        lg = logits[core * BLOC:(core + 1) * BLOC]
        sl = sent_lengths[core * BLOC:(core + 1) * BLOC]
        in_maps.append(make_core_inputs(lg, sl, consts))

    br = run_bass_kernel_spmd(nc, in_maps, core_ids=list(range(N_CORES)))
    out = np.concatenate(
        [br.results[core]["tags"] for core in range(N_CORES)], axis=0
    )
    return out.astype(np.int32)


# revision 12
# speedup vs baseline: 5.3211x; 1.0291x over previous
"""nn_CRFLayer: CRF Viterbi decode on 8 Trainium2 NeuronCores.

Data parallel over batch: each core decodes 64 of the 512 sequences.
Self-contained: hardcodes B=512, T=512, D=48, n_cores=8.

Design (v4): one fused custom-DVE op per forward step computes BOTH the
segment maxima (alpha recurrence) and the first-tie argmax backpointer
codes; backpointers stay SBUF-resident and the backward pass is a single
tiny DVE gather op per step (no PE, no ACT, no DRAM round-trip).

  Forward (all DVE, 3 ops/step):
    - CRF_FUSED_CM: hand-written 4-uop program over [128, 24 pages, 49]
      (48 prev + 1 pad).  Per page it runs a max-scan (m), a strict-
      record detector via the swap-flop complement trick (a MIN stage's
      swap register holds the running max with one-element lag, so
      isnew = [s > m_old] exactly - first-tie argmax semantics matching
      jnp.argmax bitwise), a position counter, and an argmax-code scan.
      A COUNT-gated FSM writes TWO values per page from different taps:
      element 47 emits the code (a-scan via ALU_OUT), element 48 (pad)
      emits m (via the swap register, which simultaneously resets all
      scan states for the next page).  Output lands interleaved
      (code, m) x 24 directly in bphist[:, t, :].
    - tensor_add m(strided) + emit -> ah own half; stream_shuffle
      rebuilds the full 48-wide alpha row across partition pairs.
  Layout: partitions = (b, half) interleaved (p = 2b+ch), 24 cur pages
  x 48 prev per partition, prev rotated per half with pre-rotated trans.
  Padded steps multiply trans by 0 (s0 = valid per partition): alpha
  evolves freely but bp rows at t = L become all-argmax(alpha_{L-1}),
  which re-syncs the backward chain to last_tag.

  Post-forward (2 batch ops): CRF_DECODE converts raw codes (even cols)
  to global prev-tag ids ((code-1+24h) mod 48) onto the odd cols; one
  pair-swap stream_shuffle copies the partner's decoded half onto the
  even cols.  bphist rows become interleaved [partner-tag, own-tag] x 24.

  Backward (1 op/step): CRF_GATHER: accum_out = max((iota_pp == tag) *
  bp_row) chases the pointer chain entirely on the DVE, 48 elems/step.

  Output: tags = tagsq * valid, int cast, partition-stride-2 DMA of the
  even partitions -> [64, T].

Measured: ~1.24 ms/iter on HW (baseline v2: ~1.95 ms), rel err 0.0.
"""

import numpy as np

import concourse.bass as bass
import concourse.mybir as mybir
from concourse.tile import TileContext


# ---------------------------------------------------------------------------
# custom DVE ops
# ---------------------------------------------------------------------------
def _install_segscan_patches():
    import concourse.dve_spec as dsp
    if getattr(dsp, "_crf_segscan_patched", False):
        return
    _orig_overrides = dsp._scan_overrides

    def _patched(scans, node_stage):
        seed, step = _orig_overrides(scans, node_stage)
        for sc in scans:
            if sc._subdim_step is dsp.MaxNeg:
                step[node_stage[sc]] = dsp._Stage(dsp.AluOp.BYPASS, sc.expr)
            elif sc._subdim_step is dsp.One:
                step[node_stage[sc]] = dsp._Stage(
                    sc.op, dsp._scan_init(sc), sc.expr)
        return seed, step

    dsp._scan_overrides = _patched
    _orig_nas = dsp._node_as_stage

    def _patched_nas(e):
        if isinstance(e, dsp.Scan) and (
            e._subdim_step is dsp.MaxNeg or e._subdim_step is dsp.One
        ):
            from concourse.dve_uop import AluInp as _AI
            return dsp._Stage(e.op, _AI.CURR_ALU_OUT, e.expr)
        return _orig_nas(e)

    dsp._node_as_stage = _patched_nas
    dsp._crf_segscan_patched = True


def _reg(name, spec, subdim=False, patch=None):
    import concourse.dve_ops as dvo
    from concourse.dve_spec import _has_src1, lower as dve_lower
    from concourse.dve_uop import DveOpSpec

    if name in dvo.CUSTOM_DVE_SPECS:
        return next(op for op in dvo.OPS if op.name == name)
    row = dvo._CUSTOM_DVE_ROW_BASE + len(dvo.OPS)
    shas = {}
    specs = {}
    for ver in ("v3", "v4"):
        uops = dve_lower(spec, ver=ver)
        if patch is not None:
            uops = patch(uops, ver)
        s = DveOpSpec(name=name, opcode=row, uops=uops, rd1_en=_has_src1(spec))
        shas[ver] = s.sha(ver)
        specs[ver] = s
    op = dvo.DveOp(name, spec, subdim=subdim, uops_sha=shas)
    for ver in ("v3", "v4"):
        dvo._COMPILE_CACHE[(name, ver)] = specs[ver]
    dvo.OPS.append(op)
    dvo.CUSTOM_DVE_SPECS[name] = spec
    dvo._SUB_OPCODE_FOR_NAME[name] = row
    return op


def _build_codes_uops():
    """Hand-written 5-uop program: per 49-elem page (48 real + 1 pad with
    s=-BIG), element 47 writes the first-tie argmax code (a-scan), element
    48 writes the segment max (m via delay lane 2).

    st0 sv=Src0*C0; st1 s=sv+Src1; st2 M'=min(swap,s) [swap:=running max,
    one-element lag]; st3 isnew=(M'<s); st4 m-scan; st5 jp counter;
    st6 cand=isnew*jp; st7 a-scan.
    """
    from concourse.dve_uop import (
        UopConfig, OutPath, OutSel, DelayInp, AluInp, InpSel, Trigger,
        AluOp, ENABLE, DELAY_OUT,
    )
    LANES = [InpSel.SRC_0, InpSel.CONST_0, InpSel.SRC_1, InpSel.ONE_F32,
             InpSel.MAX_NEG, InpSel.ZERO]
    PREV = AluInp.PREV_ALU_OUT
    CURR = AluInp.CURR_ALU_OUT
    D0 = AluInp.PREV_DELAY_0
    D1 = AluInp.PREV_DELAY_1
    D2 = AluInp.PREV_DELAY_2
    D3 = AluInp.PREV_DELAY_3
    D4 = AluInp.PREV_DELAY_4
    D5 = AluInp.PREV_DELAY_5

    def mk(kind):
        u = UopConfig()
        for i, sel in enumerate(LANES):
            u.enable_input(sel, i + 1)
        dp = u.datapath_config
        for st in range(8):
            dp[st].pass_through_delay(0, 1, 2, 3, 4, 5)
        seed = kind == "seed"
        wrm = kind == "wr_m"
        dp[0].enable_alu(AluOp.MULTIPLY, D0, D1)
        dp[1].enable_alu(AluOp.ADD, PREV, D2)
        # st2: M' swap stage.
        #   steady: out=min(m_old, s), swap := running max (one-elem lag)
        #   wr_m:   out=max(swap, MaxNeg)=m(47), swap := min(..)=MaxNeg (reset)
        if seed:
            dp[2].enable_alu(AluOp.BYPASS, D4, D4)
        elif wrm:
            dp[2].enable_alu(AluOp.MAX, AluInp.CURR_SWAP_OUT, D4)
        else:
            dp[2].enable_alu(AluOp.MIN, AluInp.CURR_SWAP_OUT, PREV)
        dp[2].swap_enable = ENABLE
        dp[2].enable_delay_from_src(DelayInp.PREV_ALU_OUT, 0)   # s -> lane0
        # st3: isnew = (M'out < s); wr_m: capture m(47) into lane0 instead
        dp[3].enable_alu(AluOp.IS_LT, PREV, D0)
        if wrm:
            dp[3].enable_delay_from_src(DelayInp.PREV_ALU_OUT, 0)  # m -> lane0
        # st4: m-scan; wr_m/seed: reset to MaxNeg
        if seed or wrm:
            dp[4].enable_alu(AluOp.BYPASS, D4, D4)
        else:
            dp[4].enable_alu(AluOp.MAX, CURR, D0)
        dp[4].enable_delay_from_src(DelayInp.PREV_ALU_OUT, 1)   # isnew -> lane1
        # st5: jp counter; wr_m/seed: reset to 0
        if seed or wrm:
            dp[5].enable_alu(AluOp.BYPASS, D5, D5)
        else:
            dp[5].enable_alu(AluOp.ADD, CURR, D3)
        # st6: cand = isnew * jp
        dp[6].enable_alu(AluOp.MULTIPLY, D1, PREV)
        # st7: a-scan; wr_m/seed: reset to MaxNeg
        if seed or wrm:
            dp[7].enable_alu(AluOp.BYPASS, D4, D4)
        else:
            dp[7].enable_alu(AluOp.MAX, CURR, PREV)

        SRC = Trigger.SRC_TENSOR_DONE
        SUB = Trigger.SUB_DIM_DONE
        CNT = Trigger.COUNT
        NONE = Trigger.NONE
        if kind == "seed":
            u.trigger = (CNT, NONE, NONE)
            u.next_uop = (1, 0, 0)
            u.repeat_count = 1
        elif kind == "steady47":
            u.trigger = (SRC, SUB, CNT)
            u.next_uop = (0, 1, 2)
            u.repeat_count = 47
            u.require_inp0 = u.require_inp1 = ENABLE
        elif kind == "wr_code":
            u.trigger = (SRC, SUB, CNT)
            u.next_uop = (0, 1, 3)
            u.repeat_count = 1
            u.require_inp0 = u.require_inp1 = ENABLE
            u.enable_output(OutSel.ALU_OUT, OutPath.WR0_LO)
        else:  # wr_m
            u.trigger = (SRC, SUB, NONE)
            u.next_uop = (0, 1, 0)
            u.require_inp0 = u.require_inp1 = ENABLE
            u.enable_output(DELAY_OUT[0], OutPath.WR0_LO)
        return u

    return [mk("seed"), mk("steady47"), mk("wr_code"), mk("wr_m")]


def _register_ops():
    import concourse.dve_spec as dsp
    from concourse.dve_spec import (
        Spec, Src0, Src1, C0, C1, maxx, AluOp, Bin, MaxNeg, Scan,
    )
    _install_segscan_patches()

    # ---- CRF_FUSED_CM: hand uops; spec only provides rd1_en/leaves ----
    def _ref_fused(in0, in1, c0, c1, c2):
        return (in0 * c0 + in1).astype(np.float32)

    fused_uops = _build_codes_uops()
    fused = _reg("CRF_FUSED_CM", Spec(
        body=Bin(AluOp.ADD, Bin(AluOp.MULTIPLY, Src0, C0), Src1),
        reference=_ref_fused,
    ), subdim=True, patch=lambda u, ver: fused_uops)

    # ---- CRF_GATHER: accum_out = max((Src0==C0)*Src1) ----
    def _ref_gather(in0, in1, c0, c1, c2):
        b = ((in0 == c0) * in1).astype(np.float32)
        mx = b.reshape(b.shape[0], -1).max(axis=-1, keepdims=True)
        return b, np.maximum(mx, np.float32(-3.4e38))

    gather = _reg("CRF_GATHER", Spec(
        body=Bin(AluOp.MULTIPLY, Bin(AluOp.IS_EQ, Src0, C0), Src1),
        accum=maxx, reference=_ref_gather,
    ))

    # ---- CRF_DECODE: (Src0 + C0) - ((Src0 + C0) >= C1)*C1 ----
    def _ref_decode(in0, in1, c0, c1, c2):
        u = in0 + c0
        return (u - (u >= c1) * c1).astype(np.float32)

    u_expr = Bin(AluOp.ADD, Src0, C0)
    decode = _reg("CRF_DECODE", Spec(
        body=Bin(AluOp.SUBTRACT, u_expr,
                 Bin(AluOp.MULTIPLY, Bin(AluOp.IS_GE, u_expr, C1), C1)),
        reference=_ref_decode,
    ))
    return fused, gather, decode


CRF_FUSED_CM, CRF_GATHER, CRF_DECODE = _register_ops()

AL = mybir.AluOpType
F32 = mybir.dt.float32
I32 = mybir.dt.int32

D = 48
HALF = 24
BLOC = 64
B = 512
T = 512
N_CORES = 8

PAIR_SWAP_MASK = [i ^ 1 for i in range(32)]


# ---------------------------------------------------------------------------
# host-side constant prep
# ---------------------------------------------------------------------------
def make_consts(trans: np.ndarray, T: int = T) -> dict[str, np.ndarray]:
    trans = np.asarray(trans, dtype=np.float32)
    # trans_rep[p=2b+ch] = [cur24 pages, prev48 rotated by 24*ch, + pad col]
    trans_rep = np.zeros((128, HALF, D + 1), dtype=np.float32)
    for ch in range(2):
        prev = (np.arange(D) + HALF * ch) % D
        cur = HALF * ch + np.arange(HALF)
        block = trans[prev][:, cur].T  # [cur24, prev48]
        for b in range(BLOC):
            trans_rep[2 * b + ch, :, 0:D] = block
    # iota_pp[p]: tag ids of decoded bp rows, interleaved
    #   even col 2q = partner tag, odd col 2q+1 = own tag
    iota_pp = np.zeros((128, D), dtype=np.float32)
    # iota_ah[p, q]: tag ids of ah rows laid out [own | partner]
    iota_ah = np.zeros((128, D), dtype=np.float32)
    for ch in range(2):
        own = HALF * ch + np.arange(HALF)
        par = HALF * (1 - ch) + np.arange(HALF)
        iota_pp[ch::2, 0:D:2] = par
        iota_pp[ch::2, 1:D:2] = own
        iota_ah[ch::2, 0:HALF] = own
        iota_ah[ch::2, HALF:D] = par
    # decode offset per partition: c0 = 24*ch - 1
    dec_c0 = np.zeros((128, 1), dtype=np.float32)
    dec_c0[0::2, 0] = -1.0
    dec_c0[1::2, 0] = HALF - 1.0
    neg_big = np.full((128, 1), -1e30, dtype=np.float32)
    return {
        "trans_rep": trans_rep.reshape(128, HALF * (D + 1)),
        "iota_pp": iota_pp,
        "iota_ah_m64": iota_ah - 64.0,
        "dec_c0": dec_c0,
        "neg_big": neg_big,
    }


def make_core_inputs(logits_core, sent_lengths_core, consts):
    L = np.asarray(sent_lengths_core, dtype=np.float32)
    lg = np.asarray(logits_core, dtype=np.float32)
    Tv = lg.shape[1]
    lg_il = lg.reshape(BLOC, Tv, 2, HALF).transpose(0, 2, 1, 3).reshape(
        128, Tv, HALF)
    ts = np.arange(Tv, dtype=np.float32)
    valid_nat = (ts[None, :] < L[:, None]).astype(np.float32)   # [64, T]
    valid_il = np.repeat(valid_nat, 2, axis=0)                  # [128, T]
    return dict(
        consts,
        logits_il=np.ascontiguousarray(lg_il),
        valid_il=np.ascontiguousarray(valid_il),
    )


# ---------------------------------------------------------------------------
# the kernel
# ---------------------------------------------------------------------------
def crf_kernel(tc: TileContext, outs, ins, T: int = T, CK: int = 32,
               repeat: int = 1, mode: str = "full"):
    nc = tc.nc
    logits_il = ins["logits_il"]      # [128, T, 24] dram f32
    tags_out = outs["tags"]           # [64, T] dram i32

    import contextlib
    with (
        tc.tile_pool(name="persist", bufs=1) as pp,
        tc.tile_pool(name="chunks", bufs=3) as cp,
        tc.tile_pool(name="work", bufs=2) as wp,
        tc.For_i(0, repeat, 1) if repeat > 1 else contextlib.nullcontext(),
    ):
        # ---- persistent constants / state ----
        trans_rep = pp.tile([128, HALF, D + 1], F32, tag="trans_rep")
        nc.sync.dma_start(trans_rep[:].rearrange("p a b -> p (a b)"),
                          ins["trans_rep"])
        iota_pp = pp.tile([128, D], F32, tag="iota_pp")
        nc.sync.dma_start(iota_pp[:], ins["iota_pp"])
        iota_ah_m64 = pp.tile([128, D], F32, tag="iota_ah_m64")
        nc.sync.dma_start(iota_ah_m64[:], ins["iota_ah_m64"])
        dec_c0 = pp.tile([128, 1], F32, tag="dec_c0")
        nc.sync.dma_start(dec_c0[:], ins["dec_c0"])
        valid_il = pp.tile([128, T], F32, tag="valid_il")
        nc.sync.dma_start(valid_il[:], ins["valid_il"])

        bphist = pp.tile([128, T, D], F32, tag="bphist")   # (code,m) pairs
        ah = [pp.tile([128, D + 1], F32, tag=f"ah{i}", name=f"ah{i}")
              for i in range(2)]
        for i in range(2):
            nc.sync.dma_start(ah[i][:, D:D + 1], ins["neg_big"])

        # ---- forward scan ----
        for t0 in range(0, T, CK):
            ck = min(CK, T - t0)
            emit_ch = cp.tile([128, CK, HALF], F32, tag="emit_ch")
            nc.sync.dma_start(emit_ch[:, 0:ck, :], logits_il[:, t0:t0 + ck, :])
            for t in range(t0, t0 + ck):
                k = t - t0
                cur, prv = ah[t % 2], ah[(t + 1) % 2]
                if mode == "fwd1":
                    prv = ah[0]
                if t == 0:
                    nc.vector.tensor_copy(out=cur[:, 0:HALF],
                                          in_=emit_ch[:, 0, :])
                    nc.vector.stream_shuffle(cur[:, HALF:D], cur[:, 0:HALF],
                                             mask=PAIR_SWAP_MASK)
                    continue
                alpha_b = prv[:].unsqueeze(1).broadcast_to(
                    [128, HALF, D + 1])
                # (code, m) pairs straight into bphist
                nc.vector._custom_dve(
                    CRF_FUSED_CM, out=bphist[:, t, :], in0=trans_rep[:],
                    in1=alpha_b, s0=valid_il[:, t:t + 1],
                )
                if mode == "fwd1":
                    continue
                nc.vector.tensor_add(out=cur[:, 0:HALF],
                                     in0=bphist[:, t, 1:D:2],
                                     in1=emit_ch[:, k, :])
                nc.vector.stream_shuffle(cur[:, HALF:D], cur[:, 0:HALF],
                                         mask=PAIR_SWAP_MASK)

        ah_fin = ah[0] if mode == "fwd1" else ah[(T - 1) % 2]

        skip_bwd = mode in ("fwd1", "fwd2", "fwd")
        # ---- decode codes -> global tags; shuffle partner half ----
        if not skip_bwd:
            # decode raw codes (even cols) -> own tags onto odd cols
            nc.vector._custom_dve(
                CRF_DECODE, out=bphist[:, 1:T, 1:D:2],
                in0=bphist[:, 1:T, 0:D:2],
                s0=dec_c0[:, 0:1], s1=48.0,
            )
            # partner's decoded tags onto even cols
            nc.vector.stream_shuffle(
                bphist[:, 1:T, 0:D:2], bphist[:, 1:T, 1:D:2],
                mask=PAIR_SWAP_MASK,
            )

        # ---- last_tag: first-tie argmax of final alpha ----
        tagsq = pp.tile([128, T], F32, tag="tagsq")
        amax = wp.tile([128, 1], F32, tag="amax")
        nc.vector.tensor_reduce(out=amax[:], in_=ah_fin[:, 0:D],
                                axis=mybir.AxisListType.X, op=AL.max)
        fmin0 = wp.tile([128, D], F32, tag="fmin0")
        nc.vector.scalar_tensor_tensor(
            out=fmin0[:], in0=ah_fin[:, 0:D], scalar=amax[:, 0:1],
            in1=iota_ah_m64[:], op0=AL.is_ge, op1=AL.mult,
        )
        nc.vector.tensor_reduce(out=tagsq[:, T - 1:T], in_=fmin0[:],
                                axis=mybir.AxisListType.X, op=AL.min)
        nc.vector.tensor_scalar(
            out=tagsq[:, T - 1:T], in0=tagsq[:, T - 1:T], scalar1=64.0,
            scalar2=None, op0=AL.add,
        )

        # ---- backward: one gather per step ----
        junk = pp.tile([128, D], F32, tag="junk")
        if not skip_bwd:
            for t in range(T - 2, -1, -1):
                nc.vector._custom_dve(
                    CRF_GATHER, out=junk[:], accum_out=tagsq[:, t:t + 1],
                    in0=iota_pp[:], in1=bphist[:, t + 1, :],
                    s0=tagsq[:, t + 1:t + 2],
                )

        # ---- mask + cast + store (even partitions) ----
        tags_f = pp.tile([128, T], F32, tag="tags_f")
        nc.vector.tensor_mul(out=tags_f[:], in0=tagsq[:], in1=valid_il[:])
        tags_i = pp.tile([128, T], I32, tag="tags_i")
        nc.vector.tensor_copy(out=tags_i[:], in_=tags_f[:])
        ev = tags_i[:].rearrange("(b c) t -> b c t", c=2)
        nc.sync.dma_start(tags_out, ev[:, 0, :])


# ---------------------------------------------------------------------------
# self-contained harness: build once, shard, run SPMD on 8 cores, unshard
# ---------------------------------------------------------------------------
import concourse.bacc as bacc
from concourse.bass_utils import run_bass_kernel_spmd


def _input_specs():
    return {
        "logits_il": ([128, T, HALF], F32),
        "trans_rep": ([128, HALF * (D + 1)], F32),
        "iota_pp": ([128, D], F32),
        "iota_ah_m64": ([128, D], F32),
        "dec_c0": ([128, 1], F32),
        "neg_big": ([128, 1], F32),
        "valid_il": ([128, T], F32),
    }


_NC = {}


def _build_nc(repeat: int = 1, mode: str = "full"):
    if (repeat, mode) in _NC:
        return _NC[(repeat, mode)]
    nc = bacc.Bacc(
        "TRN2",
        target_bir_lowering=False,
        debug=False,
        enable_asserts=True,
        num_devices=N_CORES,
    )
    ins = {
        name: nc.dram_tensor(name, shape, dt, kind="ExternalInput").ap()
        for name, (shape, dt) in _input_specs().items()
    }
    outs = {"tags": nc.dram_tensor("tags", [BLOC, T], I32,
                                   kind="ExternalOutput").ap()}
    with TileContext(nc) as tc:
        crf_kernel(tc, outs, ins, T=T, repeat=repeat, mode=mode)
    nc.compile()
    _NC[(repeat, mode)] = nc
    return nc


def kernel(logits, sent_lengths, crf_params):
    logits = np.asarray(logits, dtype=np.float32)
    sent_lengths = np.asarray(sent_lengths)
    consts = make_consts(crf_params, T)

    nc = _build_nc()
    in_maps = []
    for core in range(N_CORES):
        lg = logits[core * BLOC:(core + 1) * BLOC]
        sl = sent_lengths[core * BLOC:(core + 1) * BLOC]
        in_maps.append(make_core_inputs(lg, sl, consts))

    br = run_bass_kernel_spmd(nc, in_maps, core_ids=list(range(N_CORES)))
    out = np.concatenate(
        [br.results[core]["tags"] for core in range(N_CORES)], axis=0
    )
    return out.astype(np.int32)
